# revision 1
# baseline (speedup 1.0000x reference)
"""CGNN layer kernel for Trainium2 (8 NeuronCores, SPMD).

Sharding: core c owns batch b = c//2 and receiver-node half i0 = (c%2)*128.
Each core computes its (128, 128) output shard from full-j message passing.

Math (per core, b fixed):
  z[i,j,:]  = W1a x_i + W1b x_j + W1d a_ij + W1c c + b1        (pre-activation)
  s[i,:]    = sum_j mask_j * silu(z[i,j,:])
  aggr      = W2 s + b2 * (#live j)
  u         = silu(W3 [x, aggr] + b3); out = LN(x + W4 u + b4) * gamma + beta

Device layout: z kept as (h=128 partitions, j=256 free) per receiver i.
  - adj term: PE-transpose 4-receiver stacks of adj (j,r)->(r,j), masked evict,
    then K=32 row-tiled matmuls (tile_position) against replicated W1d^T.
  - x_j term: one K=128 matmul vs pre-masked x^T (same operands every i).
  - bias+silu+sum_j: single ACT op (bias port + accum_out).
  - masked-j bias pollution removed in closed form: s -= nm0 * silu(beta_i).

Scheduling notes: walrus gives compute instructions a budget of ONE semaphore
wait, and only waits arising from real data dependencies update Tile's
per-engine clock. The kernel therefore "absorbs" cross-engine production ticks
with tiny 1x1 matmuls that genuinely read one stale element of the producer
tile (into a dedicated PSUM scratch column), so every real matmul needs at
most its single PSUM-recycle wait. All MLP biases are folded into PSUM via
K=1 rank-1 matmuls of host-provided bias ROWS against a ones row, so no ACT
instruction ever waits on a DMA. All PSUM pools live for the whole program so
banks never alias across phases.
"""

import numpy as np
import ml_dtypes
ml_bf16 = ml_dtypes.bfloat16
from contextlib import ExitStack

import concourse.bass as bass
import concourse.bacc as bacc
import concourse.mybir as mybir
import concourse.tile as tile
from concourse.bass_utils import run_bass_kernel_spmd
from concourse.tile_rust import add_dep_helper

B, N, H, R = 4, 256, 128, 32
NI = 128          # receivers per core
NQ = NI // 4      # receiver quads
FP = mybir.dt.float32
BF = mybir.dt.bfloat16
EPS = 1e-5
ALU = mybir.AluOpType
ACTF = mybir.ActivationFunctionType

_cache = {}


def _order(later, earlier):
    a = later.ins if hasattr(later, "ins") else later
    b = earlier.ins if hasattr(earlier, "ins") else earlier
    add_dep_helper(a, b, sync=False, reason="pe order")


def _build_program():
    nc = bacc.Bacc()

    # ---- per-core DRAM parameters ----
    adj = nc.declare_dram_parameter("adj", [NI, N, R], FP, isOutput=False)
    x_all = nc.declare_dram_parameter("x_all", [N, H], FP, isOutput=False)
    xi = nc.declare_dram_parameter("xi", [NI, H], FP, isOutput=False)
    maskf = nc.declare_dram_parameter("maskf", [N], FP, isOutput=False)
    condrep = nc.declare_dram_parameter("condrep", [2 * H, H], FP, isOutput=False)
    w1aT = nc.declare_dram_parameter("w1aT", [H, H], FP, isOutput=False)
    w1bT = nc.declare_dram_parameter("w1bT", [H, H], BF, isOutput=False)
    w1cT = nc.declare_dram_parameter("w1cT", [2 * H, H], FP, isOutput=False)
    w1dTrep = nc.declare_dram_parameter("w1dTrep", [H, H], BF, isOutput=False)
    w2T = nc.declare_dram_parameter("w2T", [H, H], FP, isOutput=False)
    w3aT = nc.declare_dram_parameter("w3aT", [H, H], FP, isOutput=False)
    w3bT = nc.declare_dram_parameter("w3bT", [H, H], FP, isOutput=False)
    w4T = nc.declare_dram_parameter("w4T", [H, H], FP, isOutput=False)
    b1row = nc.declare_dram_parameter("b1row", [1, H], FP, isOutput=False)
    b2row = nc.declare_dram_parameter("b2row", [1, H], FP, isOutput=False)
    b3row = nc.declare_dram_parameter("b3row", [1, H], FP, isOutput=False)
    b4row = nc.declare_dram_parameter("b4row", [1, H], FP, isOutput=False)
    onesrow = nc.declare_dram_parameter("onesrow", [1, NI], FP, isOutput=False)
    identp = nc.declare_dram_parameter("identp", [H, H], FP, isOutput=False)
    gamma_rep = nc.declare_dram_parameter("gamma_rep", [H, H], FP, isOutput=False)
    beta_rep = nc.declare_dram_parameter("beta_rep", [H, H], FP, isOutput=False)
    out = nc.declare_dram_parameter("out", [NI, H], FP, isOutput=True)

    with ExitStack() as ctx:
        tc = ctx.enter_context(tile.TileContext(nc))
        const = ctx.enter_context(tc.tile_pool(name="const", bufs=1))
        persist = ctx.enter_context(tc.tile_pool(name="persist", bufs=1))
        work = ctx.enter_context(tc.tile_pool(name="work", bufs=2))
        adjbuf = ctx.enter_context(tc.tile_pool(name="adjbuf", bufs=3))
        scr = ctx.enter_context(tc.tile_pool(name="scr", bufs=3))
        # PSUM: 2 (setup/epilogue) + 4 (z) + 2 (adjT)
        pep = ctx.enter_context(tc.tile_pool(name="pep", bufs=2, space="PSUM"))
        pz = ctx.enter_context(tc.tile_pool(name="pz", bufs=4, space="PSUM"))
        pt = ctx.enter_context(tc.tile_pool(name="pt", bufs=2, space="PSUM"))

        cload_tiles = []

        def cload(ap, shape, tag, dt=FP):
            if not isinstance(ap, bass.AP):
                ap = ap[:]
            t = const.tile(shape, dt, tag=tag, name=tag)
            nc.sync.dma_start(out=t, in_=ap)
            cload_tiles.append(t)
            return t

        ident_sb = cload(identp, [H, H], "ident")
        w1aT_sb = cload(w1aT, [H, H], "w1aT")
        w1bT_sb = cload(w1bT, [H, H], "w1bT", dt=BF)
        w1cT_sb0 = cload(w1cT[0:H, :], [H, H], "w1cT0")
        w1cT_sb1 = cload(w1cT[H:2 * H, :], [H, H], "w1cT1")
        w1dTrep_sb = cload(w1dTrep, [H, H], "w1dTrep", dt=BF)
        w2T_sb = cload(w2T, [H, H], "w2T")
        w3aT_sb = cload(w3aT, [H, H], "w3aT")
        w3bT_sb = cload(w3bT, [H, H], "w3bT")
        w4T_sb = cload(w4T, [H, H], "w4T")
        condrep_sb0 = cload(condrep[0:H, :], [H, H], "condrep0")
        condrep_sb1 = cload(condrep[H:2 * H, :], [H, H], "condrep1")
        b1r_sb = cload(b1row, [1, H], "b1r")
        b2r_sb = cload(b2row, [1, H], "b2r")
        b3r_sb = cload(b3row, [1, H], "b3r")
        b4r_sb = cload(b4row, [1, H], "b4r")
        ones_sb = cload(onesrow, [1, NI], "onesr")
        xi_sb = cload(xi, [NI, H], "xi")
        xall_sb0 = cload(x_all[0:H, :], [H, H], "xall0")
        xall_sb1 = cload(x_all[H:N, :], [H, H], "xall1")
        gamma_sb = cload(gamma_rep, [H, H], "gamma_rep")
        beta_sb = cload(beta_rep, [H, H], "beta_rep")

        # mask broadcast to all partitions: (128, 256)
        maskrep = persist.tile([H, N], FP, tag="maskrep", name="maskrep")
        maskf_ap = maskf[:]
        mask_bcast = bass.AP(tensor=maskf_ap.tensor, offset=maskf_ap.offset,
                             ap=[[0, H]] + list(maskf_ap.ap))
        nc.sync.dma_start(out=maskrep, in_=mask_bcast)

        # per-partition live-count and masked-out-count of senders
        msum = persist.tile([H, 1], FP, tag="msum", name="msum")
        mrow_scr = persist.tile([H, N], FP, tag="mrow_scr", name="mrow_scr")
        nc.vector.tensor_scalar(mrow_scr, maskrep, 1.0, None,
                                ALU.mult, ALU.add, accum_out=msum)
        nm0col = persist.tile([H, 1], FP, tag="nm0col", name="nm0col")
        nc.vector.tensor_scalar(nm0col, msum, -1.0, float(N), ALU.mult, ALU.add)
        # msum replicated as a row (all partitions of msum hold the same value)
        msum_row = persist.tile([1, NI], FP, tag="msum_row", name="msum_row")
        nc.vector.tensor_scalar(msum_row, ones_sb, msum[0:1, 0:1], None,
                                ALU.mult)

        xTm = persist.tile([H, N], BF, tag="xTm", name="xTm")
        xTi = persist.tile([H, NI], FP, tag="xTi", name="xTi")
        ACb = persist.tile([H, NI], FP, tag="ACb", name="ACb")
        siluAC = persist.tile([H, NI], FP, tag="siluAC", name="siluAC")
        korr = persist.tile([H, NI], FP, tag="korr", name="korr")
        S_raw = persist.tile([H, NI], FP, tag="S_raw", name="S_raw")

        # ---- setup: x transposes, ACb ----
        for half, xall_h in ((0, xall_sb0), (1, xall_sb1)):
            pxt = pep.tile([H, H], FP, tag="ps", name="pxt")
            nc.tensor.transpose(pxt, xall_h, ident_sb)
            nc.vector.scalar_tensor_tensor(
                out=xTm[:, half * H:(half + 1) * H], in0=pxt, scalar=1.0,
                in1=maskrep[:, half * H:(half + 1) * H],
                op0=ALU.mult, op1=ALU.mult)

        pxi = pep.tile([H, H], FP, tag="ps", name="pxi")
        nc.tensor.transpose(pxi, xi_sb, ident_sb)
        nc.vector.tensor_copy(xTi, pxi)

        # ACb = W1a x_i + W1c c + b1  -> (128 h, 128 i)
        pA = pep.tile([H, NI], FP, tag="ps", name="pA")
        nc.tensor.matmul(pA, lhsT=w1aT_sb, rhs=xTi, start=True, stop=False)
        nc.tensor.matmul(pA, lhsT=w1cT_sb0, rhs=condrep_sb0,
                         start=False, stop=False)
        nc.tensor.matmul(pA, lhsT=w1cT_sb1, rhs=condrep_sb1,
                         start=False, stop=False)
        nc.tensor.matmul(pA, lhsT=b1r_sb, rhs=ones_sb,
                         start=False, stop=True)
        nc.scalar.activation(ACb, pA, ACTF.Copy)

        # korr[h,i] = nm0 * silu(ACb[h,i])
        nc.scalar.activation(siluAC, ACb, ACTF.Silu)
        nc.vector.tensor_scalar(korr, siluAC, nm0col, None, ALU.mult)

        # ---- main loop over receiver quads ----
        stacks = persist.tile([H, NQ, 2, 4, R], FP, tag="stacks",
                              name="stacks")
        for q in range(NQ):
            st0 = stacks[:, q, 0]
            st1 = stacks[:, q, 1]
            for jt, st, eng in ((0, st0, nc.sync), (1, st1, nc.scalar)):
                asrc = adj[4 * q:4 * q + 4, jt * H:(jt + 1) * H, :]
                eng.dma_start(out=st, in_=asrc.rearrange("g j r -> j g r"))

            ptile = pt.tile([H, N], FP, tag="ptile", name="ptile")
            nc.tensor.transpose(
                ptile[:, 0:H], st0.rearrange("j g r -> j (g r)"), ident_sb)
            nc.tensor.transpose(
                ptile[:, H:N], st1.rearrange("j g r -> j (g r)"), ident_sb)

            atile = adjbuf.tile([H, N], BF, tag="atile", name="atile")
            nc.vector.scalar_tensor_tensor(
                out=atile, in0=ptile, scalar=1.0, in1=maskrep,
                op0=ALU.mult, op1=ALU.mult)

            zts = []
            for g in range(4):
                zt = pz.tile([H, N], FP, tag="zt", name="zt")
                nc.tensor.matmul(zt, lhsT=w1bT_sb, rhs=xTm,
                                 start=True, stop=False)
                zts.append(zt)
            for g in range(4):
                nc.tensor.matmul(
                    zts[g], lhsT=w1dTrep_sb[32 * g:32 * g + 32, :],
                    rhs=atile[32 * g:32 * g + 32, :],
                    start=False, stop=True, tile_position=(32 * g, 0))
            for g in range(4):
                li = 4 * q + g
                sct = scr.tile([H, N], BF, tag="sct", name="sct")
                nc.scalar.activation(sct, zts[g], ACTF.Silu,
                                     bias=ACb[:, li:li + 1])
                sink = scr.tile([H, N], BF, tag="sink", name="sink")
                nc.vector.tensor_scalar(sink, sct, 1.0, None, ALU.mult,
                                        ALU.add, accum_out=S_raw[:, li:li + 1])

        # ---- epilogue ----
        S_true = persist.tile([H, NI], FP, tag="S_true", name="S_true")
        nc.vector.scalar_tensor_tensor(out=S_true, in0=S_raw, scalar=0.0,
                                       in1=korr, op0=ALU.add,
                                       op1=ALU.subtract)
        # aggr = W2 s + b2 * live_count
        pa = pep.tile([H, NI], FP, tag="ps", name="pa")
        nc.tensor.matmul(pa, lhsT=w2T_sb, rhs=S_true, start=True, stop=False)
        nc.tensor.matmul(pa, lhsT=b2r_sb, rhs=msum_row, start=False,
                         stop=True)
        aggrT = work.tile([H, NI], FP, tag="aggrT", name="aggrT")
        nc.scalar.activation(aggrT, pa, ACTF.Copy)

        pu = pep.tile([H, NI], FP, tag="ps", name="pu")
        nc.tensor.matmul(pu, lhsT=w3aT_sb, rhs=xTi, start=True, stop=False)
        nc.tensor.matmul(pu, lhsT=w3bT_sb, rhs=aggrT, start=False, stop=False)
        nc.tensor.matmul(pu, lhsT=b3r_sb, rhs=ones_sb, start=False,
                         stop=True)
        u_sb = work.tile([H, NI], FP, tag="u_sb", name="u_sb")
        nc.scalar.activation(u_sb, pu, ACTF.Silu)

        pupd = pep.tile([H, NI], FP, tag="ps", name="pupd")
        nc.tensor.matmul(pupd, lhsT=w4T_sb, rhs=u_sb, start=True, stop=False)
        nc.tensor.matmul(pupd, lhsT=b4r_sb, rhs=ones_sb, start=False,
                         stop=True)
        updT = work.tile([H, NI], FP, tag="updT", name="updT")
        nc.scalar.activation(updT, pupd, ACTF.Copy)

        py = pep.tile([NI, H], FP, tag="ps", name="py")
        nc.tensor.transpose(py, updT, ident_sb)

        y_sb = work.tile([NI, H], FP, tag="y_sb", name="y_sb")
        rowsum = work.tile([NI, 1], FP, tag="rowsum", name="rowsum")
        nc.vector.scalar_tensor_tensor(out=y_sb, in0=py, scalar=0.0,
                                       in1=xi_sb, op0=ALU.add, op1=ALU.add,
                                       accum_out=rowsum)
        negmu = work.tile([NI, 1], FP, tag="negmu", name="negmu")
        nc.vector.tensor_scalar(negmu, rowsum, -1.0 / H, None, ALU.mult)

        ysq = work.tile([NI, H], FP, tag="ysq", name="ysq")
        sumsq = work.tile([NI, 1], FP, tag="sumsq", name="sumsq")
        nc.vector.scalar_tensor_tensor(out=ysq, in0=y_sb, scalar=0.0,
                                       in1=y_sb, op0=ALU.add, op1=ALU.mult,
                                       accum_out=sumsq)
        # var + eps = sumsq/H - mu^2 + eps
        ex2 = work.tile([NI, 1], FP, tag="ex2", name="ex2")
        nc.vector.tensor_scalar(ex2, sumsq, 1.0 / H, float(EPS),
                                ALU.mult, ALU.add)
        musq = work.tile([NI, 1], FP, tag="musq", name="musq")
        nc.vector.scalar_tensor_tensor(out=musq, in0=negmu, scalar=0.0,
                                       in1=negmu, op0=ALU.add, op1=ALU.mult)
        vare = work.tile([NI, 1], FP, tag="vare", name="vare")
        nc.vector.scalar_tensor_tensor(out=vare, in0=ex2, scalar=0.0,
                                       in1=musq, op0=ALU.add,
                                       op1=ALU.subtract)
        sd = work.tile([NI, 1], FP, tag="sd", name="sd")
        nc.scalar.activation(sd, vare, ACTF.Sqrt)
        rstd = work.tile([NI, 1], FP, tag="rstd", name="rstd")
        nc.vector.reciprocal(rstd, sd)

        yn = work.tile([NI, H], FP, tag="yn", name="yn")
        nc.vector.tensor_scalar(yn, y_sb, negmu, rstd, ALU.add, ALU.mult)
        yg = work.tile([NI, H], FP, tag="yg", name="yg")
        nc.vector.scalar_tensor_tensor(out=yg, in0=yn, scalar=0.0,
                                       in1=gamma_sb, op0=ALU.add,
                                       op1=ALU.mult)
        yfin = work.tile([NI, H], FP, tag="yfin", name="yfin")
        nc.vector.scalar_tensor_tensor(out=yfin, in0=yg, scalar=0.0,
                                       in1=beta_sb, op0=ALU.add,
                                       op1=ALU.add)
        nc.sync.dma_start(out=out[:], in_=yfin)

    nc.finalize()
    return nc


def _get_program():
    if "nc" not in _cache:
        _cache["nc"] = _build_program()
    return _cache["nc"]


def kernel(x, adj_dist, mask, cond_vec, W1, b1, W2, b2, W3, b3, W4, b4,
           gamma, beta):
    x = np.asarray(x, dtype=np.float32)
    adj_dist = np.asarray(adj_dist, dtype=np.float32)
    mask_np = np.asarray(mask)
    cond_vec = np.asarray(cond_vec, dtype=np.float32)
    W1 = np.asarray(W1, dtype=np.float32)
    W2 = np.asarray(W2, dtype=np.float32)
    W3 = np.asarray(W3, dtype=np.float32)
    W4 = np.asarray(W4, dtype=np.float32)

    def c(a):
        return np.ascontiguousarray(a, dtype=np.float32)

    shared = dict(
        w1aT=c(W1[:, 0:H].T),
        w1bT=np.ascontiguousarray(W1[:, H:2 * H].T.astype(ml_bf16)),
        w1cT=c(W1[:, 2 * H + R:].T),
        w1dTrep=np.ascontiguousarray(
            np.tile(W1[:, 2 * H:2 * H + R].T, (4, 1)).astype(ml_bf16)),
        w2T=c(W2.T), w3aT=c(W3[:, 0:H].T), w3bT=c(W3[:, H:2 * H].T),
        w4T=c(W4.T),
        b1row=c(np.asarray(b1).reshape(1, H)),
        b2row=c(np.asarray(b2).reshape(1, H)),
        b3row=c(np.asarray(b3).reshape(1, H)),
        b4row=c(np.asarray(b4).reshape(1, H)),
        onesrow=c(np.ones((1, NI))),
        identp=c(np.eye(H)),
        gamma_rep=c(np.tile(np.asarray(gamma)[None, :], (H, 1))),
        beta_rep=c(np.tile(np.asarray(beta)[None, :], (H, 1))),
    )

    in_maps = []
    for core in range(8):
        b, ih = core // 2, core % 2
        i0 = ih * NI
        m = dict(shared)
        m["adj"] = c(adj_dist[b, i0:i0 + NI])
        m["x_all"] = c(x[b])
        m["xi"] = c(x[b, i0:i0 + NI])
        m["maskf"] = c(mask_np[b].astype(np.float32))
        m["condrep"] = c(np.tile(cond_vec[b][:, None], (1, H)))
        in_maps.append(m)

    nc = _get_program()
    _cache["in_maps"] = in_maps
    res = run_bass_kernel_spmd(nc, in_maps, list(range(8)))

    out_full = np.empty((B, N, H), dtype=np.float32)
    for core in range(8):
        b, ih = core // 2, core % 2
        out_full[b, ih * NI:(ih + 1) * NI] = res.results[core]["out"]
    return out_full



# revision 8
# speedup vs baseline: 1.7768x; 1.7768x over previous
"""CGNN layer kernel for Trainium2 (8 NeuronCores, SPMD) — v3.

Sharding: core c owns batch b = c//2 and receiver-node half i0 = (c%2)*128.
Each core computes its (128, 128) output shard.

Cost model learned from traces: every PE matmul costs ~173ns fixed (SBUF
access latency, mostly unoverlapped) + free_cols * 0.83ns (1.2GHz mid
p-state).  ACT ~250ns fixed + cols * 0.83ns.  DVE ~150ns fixed + cols *
~0.5-1ns.  So the design minimizes instruction count and maximizes free
size (512 cap) everywhere:

 1. Mask-packing on host: masked senders contribute nothing, so the host
    gathers live sender columns and pads to L=128 (live counts are <=126
    here; exact numpy fallback if ever exceeded).  Each 512-wide PSUM bank
    holds exactly 4 receivers' z columns — no padding waste.

 2. One K=40 matmul per 4 receivers computes BOTH the adj contraction and
    the per-receiver bias column ACb = W1a x_i + W1c c + b1: lhsT rows
    0-31 = W1d^T, rows 32-39 = bf16 hi/lo split of ACb for the group's 4
    receivers; rhs rows 32-39 are host-built {0,1} selector rows that are
    1 exactly on (receiver-block, live-j) positions.  Masked/padded j get
    z = 0 -> silu(0) = 0, so no correction terms anywhere.

 3. One K=128 matmul per 4 receivers adds the x_j broadcast W1b x_j
    (rhs = packed x^T replicated 4x).  64 main-loop matmuls total.

 4. SILU runs bias-free on 1024 columns (8 receivers, 2 PSUM banks) per
    ACT instruction.

 5. The j-reduction sum_j silu(z) is a binary tree of wide DVE
    tensor_tensor adds over the persistent silu buffer (bf16 for the
    first 3 levels, fp32 after), chunked 4x to overlap the main loop.

Epilogue: aggr = W2 S + b2*live; u = silu(W3 [x,aggr] + b3);
          out = LN(x + W4 u + b4) * gamma + beta.
"""

import numpy as np
import ml_dtypes
ml_bf16 = ml_dtypes.bfloat16
from contextlib import ExitStack

import concourse.bass as bass
import concourse.bacc as bacc
import concourse.mybir as mybir
import concourse.tile as tile
from concourse.bass_utils import run_bass_kernel_spmd

B, N, H, R = 4, 256, 128, 32
NI = 128          # receivers per core
L = 128           # padded live-sender count (mask-packed j axis)
NG = NI // 4      # receiver groups of 4 (one 512-col PSUM bank each)
NT = NG // 2      # main-loop iterations (2 groups / 8 receivers each)
FP = mybir.dt.float32
BF = mybir.dt.bfloat16
EPS = 1e-5
ALU = mybir.AluOpType
ACTF = mybir.ActivationFunctionType

_cache = {}


def _build_program():
    nc = bacc.Bacc()

    # ---- per-core DRAM parameters ----
    adjS = nc.declare_dram_parameter("adjS", [40, NG, 512], BF, isOutput=False)
    w40c = nc.declare_dram_parameter("w40c", [32, NG, H], BF, isOutput=False)
    xTm4 = nc.declare_dram_parameter("xTm4", [H, 512], BF, isOutput=False)
    xiT = nc.declare_dram_parameter("xiT", [H, NI], FP, isOutput=False)
    xi = nc.declare_dram_parameter("xi", [NI, H], FP, isOutput=False)
    krow = nc.declare_dram_parameter("krow", [1, H], FP, isOutput=False)
    liverow = nc.declare_dram_parameter("liverow", [1, NI], FP, isOutput=False)
    w1aT = nc.declare_dram_parameter("w1aT", [H, H], FP, isOutput=False)
    w1bT = nc.declare_dram_parameter("w1bT", [H, H], BF, isOutput=False)
    w2T = nc.declare_dram_parameter("w2T", [H, H], FP, isOutput=False)
    w3aT = nc.declare_dram_parameter("w3aT", [H, H], FP, isOutput=False)
    w3bT = nc.declare_dram_parameter("w3bT", [H, H], FP, isOutput=False)
    w4T = nc.declare_dram_parameter("w4T", [H, H], FP, isOutput=False)
    b2row = nc.declare_dram_parameter("b2row", [1, H], FP, isOutput=False)
    b3row = nc.declare_dram_parameter("b3row", [1, H], FP, isOutput=False)
    b4row = nc.declare_dram_parameter("b4row", [1, H], FP, isOutput=False)
    onesrow = nc.declare_dram_parameter("onesrow", [1, NI], FP, isOutput=False)
    identp = nc.declare_dram_parameter("identp", [H, H], FP, isOutput=False)
    gamma_rep = nc.declare_dram_parameter("gamma_rep", [H, H], FP, isOutput=False)
    beta_rep = nc.declare_dram_parameter("beta_rep", [H, H], FP, isOutput=False)
    out = nc.declare_dram_parameter("out", [NI, H], FP, isOutput=True)

    with ExitStack() as ctx:
        tc = ctx.enter_context(tile.TileContext(nc))
        const = ctx.enter_context(tc.tile_pool(name="const", bufs=1))
        big = ctx.enter_context(tc.tile_pool(name="big", bufs=1))
        work = ctx.enter_context(tc.tile_pool(name="work", bufs=1))
        # PSUM: 3 x 2-bank z tiles + 1 bank epilogue = 7 of 8
        pz = ctx.enter_context(tc.tile_pool(name="pz", bufs=3, space="PSUM"))
        pep = ctx.enter_context(tc.tile_pool(name="pep", bufs=1, space="PSUM"))

        def cload(eng, ap, shape, tag, dt=FP):
            if not isinstance(ap, bass.AP):
                ap = ap[:]
            t = const.tile(shape, dt, tag=tag, name=tag)
            eng.dma_start(out=t, in_=ap)
            return t

        # consts needed early for the ACb prologue (gpsimd queue)
        xiT_sb = cload(nc.gpsimd, xiT, [H, NI], "xiT")
        w1aT_sb = cload(nc.gpsimd, w1aT, [H, H], "w1aT")
        krow_sb = cload(nc.gpsimd, krow, [1, H], "krow")
        ones_sb = cload(nc.gpsimd, onesrow, [1, NI], "onesr")
        # main-loop operands (sync queue, ahead of the adjS stream)
        xTm4_sb = cload(nc.sync, xTm4, [H, 512], "xTm4", dt=BF)
        w1bT_sb = cload(nc.sync, w1bT, [H, H], "w1bT", dt=BF)
        # epilogue consts (scalar queue; ACT idle during prologue)
        w2T_sb = cload(nc.scalar, w2T, [H, H], "w2T")
        w3aT_sb = cload(nc.scalar, w3aT, [H, H], "w3aT")
        w3bT_sb = cload(nc.scalar, w3bT, [H, H], "w3bT")
        w4T_sb = cload(nc.scalar, w4T, [H, H], "w4T")
        b2r_sb = cload(nc.scalar, b2row, [1, H], "b2r")
        b3r_sb = cload(nc.scalar, b3row, [1, H], "b3r")
        b4r_sb = cload(nc.scalar, b4row, [1, H], "b4r")
        liver_sb = cload(nc.scalar, liverow, [1, NI], "liver")
        identp_sb = cload(nc.scalar, identp, [H, H], "identp")
        gamma_sb = cload(nc.scalar, gamma_rep, [H, H], "gamma_rep")
        beta_sb = cload(nc.scalar, beta_rep, [H, H], "beta_rep")
        xi_sb = cload(nc.scalar, xi, [NI, H], "xi")

        # big tiles
        W40 = big.tile([40, NG, H], BF, tag="W40", name="W40")
        nc.gpsimd.dma_start(out=W40[0:32, :, :], in_=w40c[:])
        adjS_sb = big.tile([40, NG, 512], BF, tag="adjS", name="adjS_sb")
        for c in range(4):
            nc.sync.dma_start(out=adjS_sb[:, 8 * c:8 * (c + 1), :],
                              in_=adjS[:, 8 * c:8 * (c + 1), :])
        silu_all = big.tile([H, NI * L], BF, tag="silu_all", name="silu_all")
        T1 = big.tile([H, NI * 64], BF, tag="T1", name="T1")
        T2 = big.tile([H, NI * 32], BF, tag="T2", name="T2")
        T3 = big.tile([H, NI * 16], BF, tag="T3", name="T3")
        T4 = big.tile([H, NI * 8], FP, tag="T4", name="T4")
        T5 = big.tile([H, NI * 4], FP, tag="T5", name="T5")
        T6 = big.tile([H, NI * 2], FP, tag="T6", name="T6")
        S_sb = big.tile([H, NI], FP, tag="S_sb", name="S_sb")

        # ---- prologue: ACbT = (W1a x_i + W1c c + b1)^T, hi/lo bf16 split ----
        pacb = pep.tile([NI, H], FP, tag="ps", name="pacb")
        nc.tensor.matmul(pacb, lhsT=xiT_sb, rhs=w1aT_sb, start=True,
                         stop=False)
        nc.tensor.matmul(pacb, lhsT=ones_sb, rhs=krow_sb, start=False,
                         stop=True)
        acb_hi = work.tile([NI, H], BF, tag="acb_hi", name="acb_hi")
        nc.scalar.activation(acb_hi, pacb, ACTF.Copy)
        acb_lo = work.tile([NI, H], BF, tag="acb_lo", name="acb_lo")
        nc.vector.scalar_tensor_tensor(out=acb_lo, in0=pacb, scalar=0.0,
                                       in1=acb_hi, op0=ALU.add,
                                       op1=ALU.subtract)
        # scatter ACb rows into W40[32 + 2u + v] (receiver 4g+u -> group g)
        for u in range(4):
            for v, src in ((0, acb_hi), (1, acb_lo)):
                src_ap = bass.AP(tensor=src.tensor,
                                 offset=src.offset + u * H,
                                 ap=[[4 * H, 32], [1, H]])
                nc.gpsimd.dma_start(
                    out=W40[32 + 2 * u + v:33 + 2 * u + v, :, :],
                    in_=src_ap)

        # ---- main loop: 8 receivers (2 groups, 2 PSUM banks) per iter ----
        def tree_chunk(c):
            """Reduce receivers 32c..32c+31 (silu cols 4096c..+4096)."""
            def vw(t, percv, off_elems, width):
                return bass.AP(tensor=t.tensor, offset=t.offset + off_elems,
                               ap=[list(t.ap)[0], [percv, 32], [1, width]])
            specs = [
                (silu_all, 128, T1, 64),
                (T1, 64, T2, 32),
                (T2, 32, T3, 16),
                (T3, 16, T4, 8),
                (T4, 8, T5, 4),
                (T5, 4, T6, 2),
            ]
            for src, sw, dst, dw in specs:
                nc.vector.tensor_tensor(
                    out=vw(dst, dw, 32 * c * dw, dw),
                    in0=vw(src, sw, 32 * c * sw, dw),
                    in1=vw(src, sw, 32 * c * sw + dw, dw),
                    op=ALU.add)
            nc.vector.tensor_tensor(
                out=vw(S_sb, 1, 32 * c, 1),
                in0=vw(T6, 2, 64 * c, 1),
                in1=vw(T6, 2, 64 * c + 1, 1),
                op=ALU.add)

        for t in range(NT):
            zt = pz.tile([H, 1024], FP, tag="zt", name="zt")
            for gi in range(2):
                g = 2 * t + gi
                c0 = 512 * gi
                nc.tensor.matmul(zt[:, c0:c0 + 512], lhsT=W40[:, g, :],
                                 rhs=adjS_sb[:, g, :], start=True,
                                 stop=False)
                nc.tensor.matmul(zt[:, c0:c0 + 512], lhsT=w1bT_sb,
                                 rhs=xTm4_sb, start=False, stop=True)
            nc.scalar.activation(silu_all[:, 1024 * t:1024 * (t + 1)], zt,
                                 ACTF.Silu)
            if t % 4 == 3:
                tree_chunk(t // 4)

        # ---- epilogue ----
        pa = pep.tile([H, NI], FP, tag="ps", name="pa")
        nc.tensor.matmul(pa, lhsT=w2T_sb, rhs=S_sb, start=True, stop=False)
        nc.tensor.matmul(pa, lhsT=b2r_sb, rhs=liver_sb, start=False,
                         stop=True)
        aggrT = work.tile([H, NI], FP, tag="aggrT", name="aggrT")
        nc.scalar.activation(aggrT, pa, ACTF.Copy)

        pu = pep.tile([H, NI], FP, tag="ps", name="pu")
        nc.tensor.matmul(pu, lhsT=w3aT_sb, rhs=xiT_sb, start=True, stop=False)
        nc.tensor.matmul(pu, lhsT=w3bT_sb, rhs=aggrT, start=False, stop=False)
        nc.tensor.matmul(pu, lhsT=b3r_sb, rhs=ones_sb, start=False,
                         stop=True)
        u_sb = work.tile([H, NI], FP, tag="u_sb", name="u_sb")
        nc.scalar.activation(u_sb, pu, ACTF.Silu)

        pupd = pep.tile([H, NI], FP, tag="ps", name="pupd")
        nc.tensor.matmul(pupd, lhsT=w4T_sb, rhs=u_sb, start=True, stop=False)
        nc.tensor.matmul(pupd, lhsT=b4r_sb, rhs=ones_sb, start=False,
                         stop=True)
        updT = work.tile([H, NI], FP, tag="updT", name="updT")
        nc.scalar.activation(updT, pupd, ACTF.Copy)

        py = pep.tile([NI, H], FP, tag="ps", name="py")
        nc.tensor.transpose(py, updT, identp_sb)

        y_sb = work.tile([NI, H], FP, tag="y_sb", name="y_sb")
        rowsum = work.tile([NI, 1], FP, tag="rowsum", name="rowsum")
        nc.vector.scalar_tensor_tensor(out=y_sb, in0=py, scalar=0.0,
                                       in1=xi_sb, op0=ALU.add, op1=ALU.add,
                                       accum_out=rowsum)
        negmu = work.tile([NI, 1], FP, tag="negmu", name="negmu")
        nc.vector.tensor_scalar(negmu, rowsum, -1.0 / H, None, ALU.mult)

        ysq = work.tile([NI, H], FP, tag="ysq", name="ysq")
        sumsq = work.tile([NI, 1], FP, tag="sumsq", name="sumsq")
        nc.vector.scalar_tensor_tensor(out=ysq, in0=y_sb, scalar=0.0,
                                       in1=y_sb, op0=ALU.add, op1=ALU.mult,
                                       accum_out=sumsq)
        ex2 = work.tile([NI, 1], FP, tag="ex2", name="ex2")
        nc.vector.tensor_scalar(ex2, sumsq, 1.0 / H, float(EPS),
                                ALU.mult, ALU.add)
        musq = work.tile([NI, 1], FP, tag="musq", name="musq")
        nc.vector.scalar_tensor_tensor(out=musq, in0=negmu, scalar=0.0,
                                       in1=negmu, op0=ALU.add, op1=ALU.mult)
        vare = work.tile([NI, 1], FP, tag="vare", name="vare")
        nc.vector.scalar_tensor_tensor(out=vare, in0=ex2, scalar=0.0,
                                       in1=musq, op0=ALU.add,
                                       op1=ALU.subtract)
        sd = work.tile([NI, 1], FP, tag="sd", name="sd")
        nc.scalar.activation(sd, vare, ACTF.Sqrt)
        rstd = work.tile([NI, 1], FP, tag="rstd", name="rstd")
        nc.vector.reciprocal(rstd, sd)

        yn = work.tile([NI, H], FP, tag="yn", name="yn")
        nc.vector.tensor_scalar(yn, y_sb, negmu, rstd, ALU.add, ALU.mult)
        yg = work.tile([NI, H], FP, tag="yg", name="yg")
        nc.vector.scalar_tensor_tensor(out=yg, in0=yn, scalar=0.0,
                                       in1=gamma_sb, op0=ALU.add,
                                       op1=ALU.mult)
        yfin = work.tile([NI, H], FP, tag="yfin", name="yfin")
        nc.vector.scalar_tensor_tensor(out=yfin, in0=yg, scalar=0.0,
                                       in1=beta_sb, op0=ALU.add,
                                       op1=ALU.add)
        nc.sync.dma_start(out=out[:], in_=yfin)

    nc.finalize()
    return nc


def _get_program():
    if "nc" not in _cache:
        _cache["nc"] = _build_program()
    return _cache["nc"]


def _numpy_ref(x, adj_dist, mask, cond_vec, W1, b1, W2, b2, W3, b3, W4, b4,
               gamma, beta):
    """Exact reference fallback (only used if a mask has > L live senders)."""
    b, n, h = x.shape
    out = np.empty((b, n, h), dtype=np.float32)
    for bi in range(b):
        xi_ = x[bi]
        m = mask[bi].astype(np.float32)
        base = xi_ @ W1[:, :h].T + cond_vec[bi] @ W1[:, 2 * h + R:].T + b1
        xj_t = xi_ @ W1[:, h:2 * h].T
        s = np.zeros((n, h), dtype=np.float64)
        for j in range(n):
            if m[j] == 0:
                continue
            zj = base + xj_t[j] + adj_dist[bi, :, j, :] @ W1[:, 2 * h:2 * h + R].T
            s += zj / (1 + np.exp(-zj))
        aggr = s @ W2.T + b2 * m.sum()
        u = np.concatenate([xi_, aggr], axis=1) @ W3.T + b3
        u = u / (1 + np.exp(-u))
        y = xi_ + u @ W4.T + b4
        mu = y.mean(axis=1, keepdims=True)
        var = ((y - mu) ** 2).mean(axis=1, keepdims=True)
        out[bi] = ((y - mu) / np.sqrt(var + EPS) * gamma + beta).astype(
            np.float32)
    return out


def kernel(x, adj_dist, mask, cond_vec, W1, b1, W2, b2, W3, b3, W4, b4,
           gamma, beta):
    x = np.asarray(x, dtype=np.float32)
    adj_dist = np.asarray(adj_dist, dtype=np.float32)
    mask_np = np.asarray(mask)
    cond_vec = np.asarray(cond_vec, dtype=np.float32)
    W1 = np.asarray(W1, dtype=np.float32)
    W2 = np.asarray(W2, dtype=np.float32)
    W3 = np.asarray(W3, dtype=np.float32)
    W4 = np.asarray(W4, dtype=np.float32)
    b1 = np.asarray(b1, dtype=np.float32)
    b2 = np.asarray(b2, dtype=np.float32)
    b3 = np.asarray(b3, dtype=np.float32)
    b4 = np.asarray(b4, dtype=np.float32)
    gamma = np.asarray(gamma, dtype=np.float32)
    beta = np.asarray(beta, dtype=np.float32)

    lives = [np.where(mask_np[b] != 0)[0] for b in range(B)]
    if max(len(lv) for lv in lives) > L:
        return _numpy_ref(x, adj_dist, mask_np, cond_vec, W1, b1, W2, b2,
                          W3, b3, W4, b4, gamma, beta)

    def c(a):
        return np.ascontiguousarray(a, dtype=np.float32)

    def cb(a):
        return np.ascontiguousarray(np.asarray(a).astype(ml_bf16))

    w40c_h = np.zeros((32, NG, H), dtype=ml_bf16)
    w40c_h[:, :, :] = W1[:, 2 * H:2 * H + R].T.astype(ml_bf16)[:, None, :]
    shared = dict(
        w40c=w40c_h,
        w1aT=c(W1[:, 0:H].T),
        w1bT=cb(W1[:, H:2 * H].T),
        w2T=c(W2.T), w3aT=c(W3[:, 0:H].T), w3bT=c(W3[:, H:2 * H].T),
        w4T=c(W4.T),
        b2row=c(b2.reshape(1, H)),
        b3row=c(b3.reshape(1, H)),
        b4row=c(b4.reshape(1, H)),
        onesrow=c(np.ones((1, NI))),
        identp=c(np.eye(H)),
        gamma_rep=c(np.tile(gamma[None, :], (H, 1))),
        beta_rep=c(np.tile(beta[None, :], (H, 1))),
    )
    W1c = W1[:, 2 * H + R:]
    w1d_T = W1[:, 2 * H:2 * H + R].T                      # (R, H)

    in_maps = []
    for core in range(8):
        b, ih = core // 2, core % 2
        i0 = ih * NI
        live = lives[b]
        Lv = len(live)
        m = dict(shared)
        # adjS: rows 0-31 = adj (r, (u, j-packed)); rows 32-39 = selectors
        adjS_h = np.zeros((40, NG, 512), dtype=ml_bf16)
        ablk = adj_dist[b, i0:i0 + NI][:, live, :]        # (NI, Lv, R)
        art = ablk.transpose(2, 0, 1).astype(ml_bf16)     # (R, NI, Lv)
        av = adjS_h[0:32].reshape(32, NG, 4, 128)
        av[:, :, :, 0:Lv] = art.reshape(32, NG, 4, Lv)
        sel = adjS_h[32:40].reshape(8, NG, 4, 128)
        for u in range(4):
            sel[2 * u:2 * u + 2, :, u, 0:Lv] = np.float32(1.0)
        m["adjS"] = adjS_h
        # x packed (masked by selection), transposed, replicated 4x
        xp = np.zeros((H, 512), dtype=ml_bf16)
        xt = x[b][live].T.astype(ml_bf16)                 # (H, Lv)
        for u in range(4):
            xp[:, u * L:u * L + Lv] = xt
        m["xTm4"] = xp
        m["xiT"] = c(x[b, i0:i0 + NI].T)
        m["xi"] = c(x[b, i0:i0 + NI])
        m["krow"] = c((W1c @ cond_vec[b] + b1).reshape(1, H))
        m["liverow"] = c(np.full((1, NI), float(Lv)))
        in_maps.append(m)

    nc = _get_program()
    _cache["in_maps"] = in_maps
    res = run_bass_kernel_spmd(nc, in_maps, list(range(8)))

    out_full = np.empty((B, N, H), dtype=np.float32)
    for core in range(8):
        b, ih = core // 2, core % 2
        out_full[b, ih * NI:(ih + 1) * NI] = res.results[core]["out"]
    return out_full


# revision 10
# speedup vs baseline: 2.0165x; 1.1349x over previous
"""CGNN layer kernel for Trainium2 (8 NeuronCores, SPMD) — v4.

Sharding: core c owns batch b = c//2 and receiver-node half i0 = (c%2)*128.
Each core computes its (128, 128) output shard.

Measured HW cost model (from v2/v3 traces): PE matmul = 173ns fixed +
free_cols * 0.83ns (1.2GHz); ACT = ~260ns fixed + cols * 0.83ns; DVE =
~150ns fixed + cols * ~0.5-1ns; every DMA *issue* occupies its queue
~600-1000ns.  The design minimizes instruction count everywhere:

 1. Mask-packing on host: masked senders contribute nothing, so the host
    gathers live sender columns and pads to L=128 (live counts <=126 here;
    exact numpy fallback otherwise).  A 512-col PSUM bank holds exactly 4
    receivers — zero padding waste.

 2. One K=40 matmul per 4 receivers computes the adj contraction AND the
    per-receiver bias ACb = W1a x_i + W1c c + b1: lhsT rows 0-31 = W1d^T,
    rows 32-39 = bf16 hi/lo splits of ACb, rhs rows 32-39 host-built {0,1}
    selectors over (receiver-block, live-j).  Masked/padded j get z = 0
    exactly, so silu contributes nothing — no correction terms.

 3. One K=128 matmul per 4 receivers adds W1b x_j (rhs = packed x^T
    replicated 4x).  64 main-loop matmuls, all free=512 (the cap).

 4. Bias-free SILU over 1024 cols (8 receivers, 2 PSUM banks) per ACT op.

 5. j-reduction = binary tree of wide DVE adds over the persistent silu
    buffer (bf16 3 levels, fp32 after), finely chunked to overlap.

 6. Prologue: all constants ship in 3 blob DMAs (one per queue) instead of
    ~30 small ones; ACb scatter is a single 3D-AP DMA; ACT tables
    (Copy/Silu/Sqrt) are touched early so no load lands on the epilogue.

 7. Epilogue in bf16 (single-pass matmuls) with W3b@W2 folded on the host:
    u = silu(W3a x + (W3b W2) S + b3 + W3b b2 * live), skipping the aggr
    stage; out = LN(x + W4 u + b4) * gamma + beta in fp32 on DVE.
"""

import numpy as np
import ml_dtypes
ml_bf16 = ml_dtypes.bfloat16
from contextlib import ExitStack

import concourse.bass as bass
import concourse.bacc as bacc
import concourse.mybir as mybir
import concourse.tile as tile
from concourse.bass_utils import run_bass_kernel_spmd

B, N, H, R = 4, 256, 128, 32
NI = 128          # receivers per core
L = 128           # padded live-sender count (mask-packed j axis)
NG = NI // 4      # receiver groups of 4 (one 512-col PSUM bank each)
NT = NG // 2      # main-loop iterations (2 groups / 8 receivers each)
FP = mybir.dt.float32
BF = mybir.dt.bfloat16
EPS = 1e-5
ALU = mybir.AluOpType
ACTF = mybir.ActivationFunctionType

# bf16 blob layout (columns)
_BF_SLOTS = dict(w1bT=(0, 128), xTm4=(128, 640), identb=(640, 768),
                 xiTb=(768, 896), w32Tb=(896, 1024), w3aTb=(1024, 1152),
                 w4Tb=(1152, 1280), b3eb=(1280, 1408), b4rb=(1408, 1536),
                 onesb=(1536, 1664))
BFW = 1664

_cache = {}


def _build_program():
    nc = bacc.Bacc()

    adjS = nc.declare_dram_parameter("adjS", [40, NG, 512], BF, isOutput=False)
    w40c = nc.declare_dram_parameter("w40c", [32, NG, H], BF, isOutput=False)
    earlyb = nc.declare_dram_parameter("earlyb", [H, 512], FP, isOutput=False)
    bfb = nc.declare_dram_parameter("bfb", [H, BFW], BF, isOutput=False)
    lateb = nc.declare_dram_parameter("lateb", [H, 384], FP, isOutput=False)
    out = nc.declare_dram_parameter("out", [NI, H], FP, isOutput=True)

    with ExitStack() as ctx:
        tc = ctx.enter_context(tile.TileContext(nc))
        const = ctx.enter_context(tc.tile_pool(name="const", bufs=1))
        big = ctx.enter_context(tc.tile_pool(name="big", bufs=1))
        work = ctx.enter_context(tc.tile_pool(name="work", bufs=1))
        pz = ctx.enter_context(tc.tile_pool(name="pz", bufs=3, space="PSUM"))
        pep = ctx.enter_context(tc.tile_pool(name="pep", bufs=1, space="PSUM"))

        # ---- three const blobs, one per DMA queue ----
        early = const.tile([H, 512], FP, tag="early", name="early")
        nc.gpsimd.dma_start(out=early, in_=earlyb[:])
        xiT_sb = early[:, 0:128]
        w1aT_sb = early[:, 128:256]
        krow_sb = early[0:1, 256:384]
        ones_sb = early[0:1, 384:512]

        bft = const.tile([H, BFW], BF, tag="bft", name="bft")
        nc.sync.dma_start(out=bft, in_=bfb[:])
        sl = {}
        for k, (a, b) in _BF_SLOTS.items():
            sl[k] = bft[0:1, a:b] if k in ("b3eb", "b4rb", "onesb") \
                else bft[:, a:b]

        late = const.tile([H, 384], FP, tag="late", name="late")
        nc.scalar.dma_start(out=late, in_=lateb[:])
        xi_sb = late[:, 0:128]
        gamma_sb = late[:, 128:256]
        beta_sb = late[:, 256:384]

        W40 = big.tile([40, NG, H], BF, tag="W40", name="W40")
        nc.sync.dma_start(out=W40[0:32, :, :], in_=w40c[:])
        adjS_sb = big.tile([40, NG, 512], BF, tag="adjS", name="adjS_sb")
        for c in range(2):
            nc.sync.dma_start(out=adjS_sb[:, 16 * c:16 * (c + 1), :],
                              in_=adjS[:, 16 * c:16 * (c + 1), :])
        silu_all = big.tile([H, NI * L], BF, tag="silu_all", name="silu_all")
        T1 = big.tile([H, NI * 64], BF, tag="T1", name="T1")
        T2 = big.tile([H, NI * 32], BF, tag="T2", name="T2")
        T3 = big.tile([H, NI * 16], BF, tag="T3", name="T3")
        T4 = big.tile([H, NI * 8], FP, tag="T4", name="T4")
        T5 = big.tile([H, NI * 4], FP, tag="T5", name="T5")
        T6 = big.tile([H, NI * 2], FP, tag="T6", name="T6")
        S_bf = big.tile([H, NI], BF, tag="S_bf", name="S_bf")

        # ---- prologue: ACbT = (W1a x_i + W1c c + b1)^T, hi/lo bf16 split ----
        pacb = pep.tile([NI, H], FP, tag="ps", name="pacb")
        nc.tensor.matmul(pacb, lhsT=xiT_sb, rhs=w1aT_sb, start=True,
                         stop=False)
        nc.tensor.matmul(pacb, lhsT=ones_sb, rhs=krow_sb, start=False,
                         stop=True)
        acb_hi = work.tile([NI, H], BF, tag="acb_hi", name="acb_hi")
        nc.scalar.activation(acb_hi, pacb, ACTF.Copy)
        acb_lo = work.tile([NI, H], BF, tag="acb_lo", name="acb_lo")
        nc.vector.scalar_tensor_tensor(out=acb_lo, in0=pacb, scalar=0.0,
                                       in1=acb_hi, op0=ALU.add,
                                       op1=ALU.subtract)
        # preload the Silu and Sqrt ACT tables off the critical path
        tdum = work.tile([1, 8], FP, tag="tdum", name="tdum")
        nc.scalar.activation(tdum, ones_sb[0:1, 0:8], ACTF.Silu)
        nc.scalar.activation(tdum, ones_sb[0:1, 0:8], ACTF.Sqrt)
        # scatter ACb rows: receiver 4g+u -> W40 row 32+u (hi) / 36+u (lo)
        for base, src, eng in ((32, acb_hi, nc.gpsimd), (36, acb_lo, nc.sync)):
            for u in range(4):
                src_ap = bass.AP(tensor=src.tensor,
                                 offset=src.offset + u * H,
                                 ap=[[4 * H, 32], [1, H]])
                eng.dma_start(out=W40[base + u:base + u + 1, :, :],
                              in_=src_ap)

        # ---- main loop: 8 receivers (2 groups, 2 PSUM banks) per iter ----
        def vw(t, percv, off_elems, width, nrecv=32):
            return bass.AP(tensor=t.tensor, offset=t.offset + off_elems,
                           ap=[list(t.ap)[0], [percv, nrecv], [1, width]])

        def tree_rest(c):
            """Levels 2-7 for receivers 32c..32c+31 (after lvl1 chunks)."""
            specs = [(T1, 64, T2, 32), (T2, 32, T3, 16), (T3, 16, T4, 8),
                     (T4, 8, T5, 4), (T5, 4, T6, 2)]
            for src, sw, dst, dw in specs:
                nc.vector.tensor_tensor(
                    out=vw(dst, dw, 32 * c * dw, dw),
                    in0=vw(src, sw, 32 * c * sw, dw),
                    in1=vw(src, sw, 32 * c * sw + dw, dw),
                    op=ALU.add)
            nc.vector.tensor_tensor(
                out=vw(S_bf, 1, 32 * c, 1),
                in0=vw(T6, 2, 64 * c, 1),
                in1=vw(T6, 2, 64 * c + 1, 1),
                op=ALU.add)

        for t in range(NT):
            zt = pz.tile([H, 1024], FP, tag="zt", name="zt")
            for gi in range(2):
                g = 2 * t + gi
                c0 = 512 * gi
                nc.tensor.matmul(zt[:, c0:c0 + 512], lhsT=sl["w1bT"],
                                 rhs=sl["xTm4"], start=True, stop=False)
                nc.tensor.matmul(zt[:, c0:c0 + 512], lhsT=W40[:, g, :],
                                 rhs=adjS_sb[:, g, :], start=False,
                                 stop=True)
            nc.scalar.activation(silu_all[:, 1024 * t:1024 * (t + 1)], zt,
                                 ACTF.Silu)
            if t % 2 == 1:
                k = t // 2  # tree level 1 per 2 tiles (16 receivers)
                nc.vector.tensor_tensor(
                    out=vw(T1, 64, 1024 * k, 64, nrecv=16),
                    in0=vw(silu_all, 128, 2048 * k, 64, nrecv=16),
                    in1=vw(silu_all, 128, 2048 * k + 64, 64, nrecv=16),
                    op=ALU.add)
            if t % 4 == 3:
                tree_rest(t // 4)

        # ---- epilogue (bf16 matmuls; W3b@W2 pre-folded on host) ----
        pu = pep.tile([H, NI], FP, tag="ps", name="pu")
        nc.tensor.matmul(pu, lhsT=sl["w3aTb"], rhs=sl["xiTb"], start=True,
                         stop=False)
        nc.tensor.matmul(pu, lhsT=sl["w32Tb"], rhs=S_bf, start=False,
                         stop=False)
        nc.tensor.matmul(pu, lhsT=sl["b3eb"], rhs=sl["onesb"], start=False,
                         stop=True)
        u_sb = work.tile([H, NI], BF, tag="u_sb", name="u_sb")
        nc.scalar.activation(u_sb, pu, ACTF.Silu)

        pupd = pep.tile([H, NI], FP, tag="ps", name="pupd")
        nc.tensor.matmul(pupd, lhsT=sl["w4Tb"], rhs=u_sb, start=True,
                         stop=False)
        nc.tensor.matmul(pupd, lhsT=sl["b4rb"], rhs=sl["onesb"],
                         start=False, stop=True)
        updT = work.tile([H, NI], BF, tag="updT", name="updT")
        nc.scalar.activation(updT, pupd, ACTF.Copy)

        py = pep.tile([NI, H], BF, tag="ps", name="py")
        nc.tensor.transpose(py, updT, sl["identb"])

        y_sb = work.tile([NI, H], FP, tag="y_sb", name="y_sb")
        rowsum = work.tile([NI, 1], FP, tag="rowsum", name="rowsum")
        nc.vector.scalar_tensor_tensor(out=y_sb, in0=py, scalar=0.0,
                                       in1=xi_sb, op0=ALU.add, op1=ALU.add,
                                       accum_out=rowsum)
        negmu = work.tile([NI, 1], FP, tag="negmu", name="negmu")
        nc.vector.tensor_scalar(negmu, rowsum, -1.0 / H, None, ALU.mult)

        ysq = work.tile([NI, H], FP, tag="ysq", name="ysq")
        sumsq = work.tile([NI, 1], FP, tag="sumsq", name="sumsq")
        nc.vector.scalar_tensor_tensor(out=ysq, in0=y_sb, scalar=0.0,
                                       in1=y_sb, op0=ALU.add, op1=ALU.mult,
                                       accum_out=sumsq)
        ex2 = work.tile([NI, 1], FP, tag="ex2", name="ex2")
        nc.vector.tensor_scalar(ex2, sumsq, 1.0 / H, float(EPS),
                                ALU.mult, ALU.add)
        musq = work.tile([NI, 1], FP, tag="musq", name="musq")
        nc.vector.scalar_tensor_tensor(out=musq, in0=negmu, scalar=0.0,
                                       in1=negmu, op0=ALU.add, op1=ALU.mult)
        vare = work.tile([NI, 1], FP, tag="vare", name="vare")
        nc.vector.scalar_tensor_tensor(out=vare, in0=ex2, scalar=0.0,
                                       in1=musq, op0=ALU.add,
                                       op1=ALU.subtract)
        sd = work.tile([NI, 1], FP, tag="sd", name="sd")
        nc.scalar.activation(sd, vare, ACTF.Sqrt)
        rstd = work.tile([NI, 1], FP, tag="rstd", name="rstd")
        nc.vector.reciprocal(rstd, sd)

        yn = work.tile([NI, H], FP, tag="yn", name="yn")
        nc.vector.tensor_scalar(yn, y_sb, negmu, rstd, ALU.add, ALU.mult)
        yg = work.tile([NI, H], FP, tag="yg", name="yg")
        nc.vector.scalar_tensor_tensor(out=yg, in0=yn, scalar=0.0,
                                       in1=gamma_sb, op0=ALU.add,
                                       op1=ALU.mult)
        yfin = work.tile([NI, H], FP, tag="yfin", name="yfin")
        nc.vector.scalar_tensor_tensor(out=yfin, in0=yg, scalar=0.0,
                                       in1=beta_sb, op0=ALU.add,
                                       op1=ALU.add)
        nc.sync.dma_start(out=out[:], in_=yfin)

    nc.finalize()
    return nc


def _get_program():
    if "nc" not in _cache:
        _cache["nc"] = _build_program()
    return _cache["nc"]


def _numpy_ref(x, adj_dist, mask, cond_vec, W1, b1, W2, b2, W3, b3, W4, b4,
               gamma, beta):
    """Exact reference fallback (only used if a mask has > L live senders)."""
    b, n, h = x.shape
    out = np.empty((b, n, h), dtype=np.float32)
    for bi in range(b):
        xi_ = x[bi]
        m = mask[bi].astype(np.float32)
        base = xi_ @ W1[:, :h].T + cond_vec[bi] @ W1[:, 2 * h + R:].T + b1
        xj_t = xi_ @ W1[:, h:2 * h].T
        s = np.zeros((n, h), dtype=np.float64)
        for j in range(n):
            if m[j] == 0:
                continue
            zj = base + xj_t[j] + adj_dist[bi, :, j, :] @ W1[:, 2 * h:2 * h + R].T
            s += zj / (1 + np.exp(-zj))
        aggr = s @ W2.T + b2 * m.sum()
        u = np.concatenate([xi_, aggr], axis=1) @ W3.T + b3
        u = u / (1 + np.exp(-u))
        y = xi_ + u @ W4.T + b4
        mu = y.mean(axis=1, keepdims=True)
        var = ((y - mu) ** 2).mean(axis=1, keepdims=True)
        out[bi] = ((y - mu) / np.sqrt(var + EPS) * gamma + beta).astype(
            np.float32)
    return out


def kernel(x, adj_dist, mask, cond_vec, W1, b1, W2, b2, W3, b3, W4, b4,
           gamma, beta):
    x = np.asarray(x, dtype=np.float32)
    adj_dist = np.asarray(adj_dist, dtype=np.float32)
    mask_np = np.asarray(mask)
    cond_vec = np.asarray(cond_vec, dtype=np.float32)
    W1 = np.asarray(W1, dtype=np.float32)
    W2 = np.asarray(W2, dtype=np.float32)
    W3 = np.asarray(W3, dtype=np.float32)
    W4 = np.asarray(W4, dtype=np.float32)
    b1 = np.asarray(b1, dtype=np.float32)
    b2 = np.asarray(b2, dtype=np.float32)
    b3 = np.asarray(b3, dtype=np.float32)
    b4 = np.asarray(b4, dtype=np.float32)
    gamma = np.asarray(gamma, dtype=np.float32)
    beta = np.asarray(beta, dtype=np.float32)

    lives = [np.where(mask_np[b] != 0)[0] for b in range(B)]
    if max(len(lv) for lv in lives) > L:
        return _numpy_ref(x, adj_dist, mask_np, cond_vec, W1, b1, W2, b2,
                          W3, b3, W4, b4, gamma, beta)

    def c(a):
        return np.ascontiguousarray(a, dtype=np.float32)

    w40c_h = np.zeros((32, NG, H), dtype=ml_bf16)
    w40c_h[:, :, :] = W1[:, 2 * H:2 * H + R].T.astype(ml_bf16)[:, None, :]
    W32 = W3[:, H:] @ W2                                   # (H, H)
    w32b2 = W3[:, H:] @ b2                                 # (H,)

    in_maps = []
    for core in range(8):
        b, ih = core // 2, core % 2
        i0 = ih * NI
        live = lives[b]
        Lv = len(live)
        m = dict(w40c=w40c_h)
        # adjS: rows 0-31 = adj (r, (u, j-packed)); rows 32-39 = selectors
        adjS_h = np.zeros((40, NG, 512), dtype=ml_bf16)
        ablk = adj_dist[b, i0:i0 + NI][:, live, :]          # (NI, Lv, R)
        art = ablk.transpose(2, 0, 1).astype(ml_bf16)       # (R, NI, Lv)
        av = adjS_h[0:32].reshape(32, NG, 4, 128)
        av[:, :, :, 0:Lv] = art.reshape(32, NG, 4, Lv)
        sel = adjS_h[32:40].reshape(8, NG, 4, 128)
        for u in range(4):
            sel[[u, 4 + u], :, u, 0:Lv] = np.float32(1.0)
        m["adjS"] = adjS_h

        # early fp32 blob: xiT | w1aT | krow(row0) | ones(row0)
        eb = np.zeros((H, 512), dtype=np.float32)
        eb[:, 0:128] = x[b, i0:i0 + NI].T
        eb[:, 128:256] = W1[:, 0:H].T
        eb[0, 256:384] = W1[:, 2 * H + R:] @ cond_vec[b] + b1
        eb[0, 384:512] = 1.0
        m["earlyb"] = c(eb)

        # bf16 blob
        bb = np.zeros((H, BFW), dtype=ml_bf16)
        def put(key, arr):
            a0, a1 = _BF_SLOTS[key]
            bb[0:arr.shape[0], a0:a1] = arr.astype(ml_bf16)
        put("w1bT", W1[:, H:2 * H].T)
        xp = np.zeros((H, 512), dtype=np.float32)
        xt = x[b][live].T
        for u in range(4):
            xp[:, u * L:u * L + Lv] = xt
        put("xTm4", xp)
        put("identb", np.eye(H, dtype=np.float32))
        put("xiTb", x[b, i0:i0 + NI].T)
        put("w32Tb", W32.T)
        put("w3aTb", W3[:, 0:H].T)
        put("w4Tb", W4.T)
        put("b3eb", (b3 + w32b2 * float(Lv)).reshape(1, H))
        put("b4rb", b4.reshape(1, H))
        put("onesb", np.ones((1, NI), dtype=np.float32))
        m["bfb"] = bb

        lb = np.zeros((H, 384), dtype=np.float32)
        lb[:, 0:128] = x[b, i0:i0 + NI]
        lb[:, 128:256] = np.tile(gamma[None, :], (H, 1))
        lb[:, 256:384] = np.tile(beta[None, :], (H, 1))
        m["lateb"] = c(lb)
        in_maps.append(m)

    nc = _get_program()
    _cache["in_maps"] = in_maps
    res = run_bass_kernel_spmd(nc, in_maps, list(range(8)))

    out_full = np.empty((B, N, H), dtype=np.float32)
    for core in range(8):
        b, ih = core // 2, core % 2
        out_full[b, ih * NI:(ih + 1) * NI] = res.results[core]["out"]
    return out_full
